# revision 10
# baseline (speedup 1.0000x reference)
"""Trainium2 Bass kernel for nn_BagModel_3d (segment_reduce).

Computation (per bag b):
  out[b] = (1/n_b) * sum_{i < n_b} relu(x[b, i, :] @ W1 + b1) @ W2 + b2

Strategy (8 cores, data-parallel over bags):
  * Host: sort bags by n_instances, snake-deal across cores (balanced work),
    concatenate ONLY the valid instances per core (exact compaction - the
    random n_b average ~256/512, so this halves DMA and matmul work), cast
    to bf16, zero-pad to G*128 columns.
  * Device, instance-major layout: for each 128-instance group, the x tile
    [d_in=128, inst=128] is the matmul STATIONARY operand and W1 [128, 256]
    the moving one, giving z^T [inst, dh] in PSUM.  The PSUM drain is then a
    bag-agnostic relu at FD=1024 (alternating ScalarE / VectorE) into a bf16
    h^T in SBUF - no per-bag accum pieces on the slow engines.
  * The ragged per-bag segment sum runs on TensorE: a {0,1} indicator matrix
    S[g] [128 inst, 32 bags] as stationary, h^T as moving, 4-way column
    tiling (tile_position), accumulating 4 PSUM band rows across all groups.
  * Final: one tensor_tensor_reduce contracts the pooled bands with W2 along
    the free axis, a [128,32] fold matmul adds the 4 bands per bag, then
    scale by 1/n_b and add b2 in a single tensor_scalar.
  * PE warmup: dummy matmuls during the initial DMA window so the HAM clock
    gate reaches 2.4 GHz before the real matmul stream starts.

b1 general-path note: padded columns are zeros and excluded by S, so no
relu(b1) correction is ever needed.  A nonzero b1 is folded in via an extra
K=1 matmul per group (ones-row x b1-row); the spec's b1 is all-zero so the
compiled program skips it.
"""
import os
import sys
import numpy as np

sys.path.insert(0, '/opt/trn_rl_repo')

# debug knobs (default = full-featured kernel)
DBG_NOWARM = os.environ.get('KDBG_NOWARM', '0') == '1'
DBG_NOTILEPOS = os.environ.get('KDBG_NOTILEPOS', '0') == '1'
DBG_NOPASS2 = os.environ.get('KDBG_NOPASS2', '0') == '1'
DBG_ALLSCALAR = os.environ.get('KDBG_ALLSCALAR', '0') == '1'
DBG_NOFINAL = os.environ.get('KDBG_NOFINAL', '0') == '1'
DBG_NODRAIN = os.environ.get('KDBG_NODRAIN', '0') == '1'

B, N_MAX, D_IN, D_H = 256, 512, 256, 256
N_CORES = 8
BAGS = B // N_CORES          # 32 bag slots per core
GPB = 6                      # instance groups per PSUM buffer
N_WARM = 12                  # warmup matmuls (~3us cold PE time)

_PROGRAMS = {}


def _build_program(G, b1_nonzero):
    import concourse.bacc as bacc
    import concourse.tile as tile
    from concourse import mybir

    f32 = mybir.dt.float32
    bf16 = mybir.dt.bfloat16
    Alu = mybir.AluOpType
    Act = mybir.ActivationFunctionType

    W = G * 128
    NBLK = G // GPB

    nc = bacc.Bacc("TRN2", target_bir_lowering=False, debug=False)

    xt = nc.dram_tensor("xt", [D_IN, W], bf16, kind="ExternalInput").ap()
    w1 = nc.dram_tensor("w1", [D_IN, D_H], bf16, kind="ExternalInput").ap()
    s_t = nc.dram_tensor("s_t", [128, G * BAGS], bf16, kind="ExternalInput").ap()
    w2b = nc.dram_tensor("w2b", [128, D_H], f32, kind="ExternalInput").ap()
    fold = nc.dram_tensor("fold", [128, BAGS], f32, kind="ExternalInput").ap()
    inv = nc.dram_tensor("inv", [BAGS, 1], f32, kind="ExternalInput").ap()
    bias2 = nc.dram_tensor("bias2", [BAGS, 1], f32, kind="ExternalInput").ap()
    if b1_nonzero:
        b1r = nc.dram_tensor("b1r", [1, D_H], bf16, kind="ExternalInput").ap()
    out = nc.dram_tensor("out", [BAGS, 1], f32, kind="ExternalOutput").ap()

    with tile.TileContext(nc) as tc:
        with (
            tc.tile_pool(name="const", bufs=1) as cpool,
            tc.tile_pool(name="xsb", bufs=1) as xpool,
            tc.tile_pool(name="hsb", bufs=1) as hpool,
            tc.tile_pool(name="z", bufs=2, space="PSUM") as zpool,
            tc.tile_pool(name="sps", bufs=1, space="PSUM") as spool,
        ):
            # ---- PE warmup: zeros matmuls fill the HAM activity window ----
            warm = cpool.tile([128, 256], bf16, tag="warm")
            nc.vector.memset(warm[:], 0.0)
            small = spool.tile([128, 512], f32, tag="small")
            bands = small[:, 0:D_H]            # 4 x 32 bag-band rows
            pot = spool.tile([BAGS, 1], f32, tag="pot")
            po = pot[:]
            for _ in range(0 if DBG_NOWARM else N_WARM):
                nc.tensor.matmul(small[:, 256:512], warm[:, 0:128], warm[:],
                                 start=True, stop=True, skip_group_check=True)

            # ---- constants + x prefetch (HWDGE, bf16 host-cast) ----
            w1k0 = cpool.tile([128, D_H], bf16, tag="w1k0")
            w1k1 = cpool.tile([128, D_H], bf16, tag="w1k1")
            nc.sync.dma_start(w1k0[:], w1[0:128, :])
            nc.sync.dma_start(w1k1[:], w1[128:256, :])
            xk0 = xpool.tile([128, W], bf16, tag="xk0")
            xk1 = xpool.tile([128, W], bf16, tag="xk1")
            NCH = 4
            cw = -(-W // (NCH * 128)) * 128
            bounds = [min(ci * cw, W) for ci in range(NCH + 1)]
            for ci in range(NCH):
                sl = slice(bounds[ci], bounds[ci + 1])
                if sl.start >= sl.stop:
                    continue
                nc.sync.dma_start(xk0[:, sl], xt[0:128, sl])
                nc.scalar.dma_start(xk1[:, sl], xt[128:256, sl])
            s_sb = cpool.tile([128, G * BAGS], bf16, tag="s_sb")
            nc.gpsimd.dma_start(s_sb[:], s_t[:])
            w2sb = cpool.tile([128, D_H], f32, tag="w2sb")
            nc.gpsimd.dma_start(w2sb[:], w2b[:])
            foldsb = cpool.tile([128, BAGS], f32, tag="foldsb")
            nc.gpsimd.dma_start(foldsb[:], fold[:])
            invsb = cpool.tile([BAGS, 1], f32, tag="invsb")
            nc.gpsimd.dma_start(invsb[:], inv[:])
            b2sb = cpool.tile([BAGS, 1], f32, tag="b2sb")
            nc.gpsimd.dma_start(b2sb[:], bias2[:])
            if b1_nonzero:
                onesr = cpool.tile([1, 128], bf16, tag="onesr")
                nc.vector.memset(onesr[:], 1.0)
                b1sb = cpool.tile([1, D_H], bf16, tag="b1sb")
                nc.sync.dma_start(b1sb[:], b1r[:])

            hT = hpool.tile([128, G * D_H], bf16, tag="hT")

            # ---- main loop: GPB instance groups per PSUM buffer ----
            for blk in range(NBLK):
                zb = zpool.tile([128, GPB * D_H], f32, tag="zb")
                for j in range(GPB):
                    g = GPB * blk + j
                    zsl = zb[:, D_H * j:D_H * (j + 1)]
                    xsl = slice(128 * g, 128 * (g + 1))
                    nc.tensor.matmul(zsl, xk0[:, xsl], w1k0[:],
                                     start=True, stop=False)
                    nc.tensor.matmul(zsl, xk1[:, xsl], w1k1[:],
                                     start=False, stop=not b1_nonzero)
                    if b1_nonzero:
                        nc.tensor.matmul(zsl, onesr[:], b1sb[:],
                                         start=False, stop=True)
                # relu drain, whole buffer in one big-FD instruction
                hsl = hT[:, GPB * D_H * blk:GPB * D_H * (blk + 1)]
                if DBG_NODRAIN:
                    pass
                elif DBG_ALLSCALAR or blk % 2 == 0:
                    nc.scalar.activation(hsl, zb[:], Act.Relu,
                                         bias=0.0, scale=1.0)
                else:
                    nc.vector.tensor_scalar(hsl, zb[:], 0.0, None, op0=Alu.max)
                # segment-sum matmuls: indicator stationary, 4-way col tiling
                if not DBG_NOPASS2:
                    for j in range(GPB):
                        g = GPB * blk + j
                        jt = j % 4
                        kw = {}
                        if not DBG_NOTILEPOS:
                            kw['tile_position'] = (0, 32 * jt)
                        nc.tensor.matmul(
                            bands[32 * jt:32 * (jt + 1), :],
                            s_sb[:, BAGS * g:BAGS * (g + 1)],
                            hT[:, D_H * g:D_H * (g + 1)],
                            start=(g < 4), stop=(g >= G - 4),
                            skip_group_check=True, **kw)

            # ---- W2 contraction + band fold + mean + bias ----
            scr = cpool.tile([128, D_H], f32, tag="scr")
            acc = cpool.tile([128, 1], f32, tag="acc")
            osb = cpool.tile([BAGS, 1], f32, tag="osb")
            if DBG_NOFINAL:
                nc.vector.memset(osb[:], 0.0)
            else:
                # acc[p] = sum_dh bands[p, dh] * W2[dh]  (proven stt+accum form)
                nc.vector.scalar_tensor_tensor(
                    scr[:], bands, 0.0, w2sb[:], op0=Alu.add, op1=Alu.mult,
                    accum_out=acc[:])
                nc.tensor.matmul(po, foldsb[:], acc[:], start=True, stop=True,
                                 skip_group_check=True)
                nc.vector.tensor_scalar(osb[:], po, invsb[:, 0:1], None,
                                        op0=Alu.mult)
                nc.vector.tensor_add(osb[:], osb[:], b2sb[:])
            nc.sync.dma_start(out[:], osb[:])

    nc.compile()
    return nc


def get_program(G, b1_nonzero):
    key = (int(G), bool(b1_nonzero))
    if key not in _PROGRAMS:
        _PROGRAMS[key] = _build_program(*key)
    return _PROGRAMS[key]


def _plan(n):
    """Snake-deal bags (sorted by size, desc) across cores; return
    assignment[core][slot] -> bag id and G (shared group count)."""
    order = np.argsort(-n, kind='stable')
    assignment = np.empty((N_CORES, BAGS), dtype=np.int64)
    for i, bag in enumerate(order):
        r, p = divmod(i, N_CORES)
        core = p if (r % 2 == 0) else (N_CORES - 1 - p)
        assignment[core, r] = bag
    v_max = max(int(n[assignment[c]].sum()) for c in range(N_CORES))
    G = -(-v_max // 128)
    G = -(-G // GPB) * GPB          # multiple of GPB (psum buffer / col tiles)
    return assignment, G


def make_in_maps(x, n_instances, W1, b1, W2, b2=None):
    import ml_dtypes
    bf16 = ml_dtypes.bfloat16

    x = np.asarray(x, dtype=np.float32)
    n = np.asarray(n_instances, dtype=np.int64)
    W1 = np.asarray(W1, dtype=np.float32)
    b1 = np.asarray(b1, dtype=np.float32).reshape(-1)
    W2 = np.asarray(W2, dtype=np.float32).reshape(-1)
    b2v = float(np.asarray(b2).reshape(-1)[0]) if b2 is not None else 0.0

    assignment, G = _plan(n)
    W = G * 128
    b1_nonzero = bool(np.any(b1 != 0.0))

    xflat = x.reshape(B * N_MAX, D_IN)
    w1_bf = np.ascontiguousarray(W1.astype(bf16))
    w2b = np.ascontiguousarray(
        np.broadcast_to(W2.reshape(1, D_H), (128, D_H)).astype(np.float32))
    foldm = np.zeros((128, BAGS), dtype=np.float32)
    foldm[np.arange(128), np.arange(128) % BAGS] = 1.0

    in_maps = []
    for c in range(N_CORES):
        bags = assignment[c]
        ns = n[bags]
        v = int(ns.sum())
        # gather valid instance rows: bag-major, instance-minor
        idx = np.concatenate(
            [bags[s] * N_MAX + np.arange(ns[s]) for s in range(BAGS)])
        xt = np.zeros((D_IN, W), dtype=bf16)
        xt[:, :v] = xflat[idx].T.astype(bf16)
        # indicator S: [W, 32] -> [128, G*32]
        starts = np.zeros(BAGS + 1, dtype=np.int64)
        np.cumsum(ns, out=starts[1:])
        s_full = np.zeros((W, BAGS), dtype=bf16)
        for s in range(BAGS):
            s_full[starts[s]:starts[s + 1], s] = bf16(1.0)
        s_t = np.ascontiguousarray(
            s_full.reshape(G, 128, BAGS).transpose(1, 0, 2).reshape(128, G * BAGS))
        im = {
            "xt": xt,
            "w1": w1_bf,
            "s_t": s_t,
            "w2b": w2b,
            "fold": foldm,
            "inv": (1.0 / ns.astype(np.float32)).reshape(BAGS, 1),
            "bias2": np.full((BAGS, 1), b2v, dtype=np.float32),
        }
        if b1_nonzero:
            im["b1r"] = np.ascontiguousarray(b1.reshape(1, D_H).astype(bf16))
        in_maps.append(im)
    return in_maps


def run_spmd(in_maps, b2_value=0.0, trace=False, **kwargs):
    from concourse import bass_utils
    if trace:
        # no S3 in this environment; keep trace artifacts local
        bass_utils.upload_artifacts = lambda tmpdir: tmpdir
    G = in_maps[0]["xt"].shape[1] // 128
    nc = get_program(G, "b1r" in in_maps[0])
    return bass_utils.run_bass_kernel_spmd(
        nc, in_maps, core_ids=list(range(N_CORES)), trace=trace, **kwargs)


def kernel(x, n_instances, W1, b1, W2, b2):
    n = np.asarray(n_instances, dtype=np.int64)
    assignment, _ = _plan(n)
    in_maps = make_in_maps(x, n_instances, W1, b1, W2, b2)
    res = run_spmd(in_maps)
    out = np.empty((B, 1), dtype=np.float32)
    for c in range(N_CORES):
        out[assignment[c]] = np.asarray(res.results[c]["out"],
                                        dtype=np.float32).reshape(BAGS, 1)
    return out


# revision 11
# speedup vs baseline: 1.0905x; 1.0905x over previous
"""Trainium2 Bass kernel for nn_BagModel_3d (segment_reduce).

Computation (per bag b):
  out[b] = (1/n_b) * sum_{i < n_b} relu(x[b, i, :] @ W1 + b1) @ W2 + b2

Strategy (8 cores, data-parallel over bags):
  * Host: sort bags by n_instances, snake-deal across cores (balanced work),
    concatenate ONLY the valid instances per core (exact compaction - the
    random n_b average ~256/512, so this halves DMA and matmul work), cast
    to bf16, zero-pad to G*128 columns.
  * Device, instance-major layout: for each 128-instance group, the x tile
    [d_in=128, inst=128] is the matmul STATIONARY operand and W1 [128, 256]
    the moving one, giving z^T [inst, dh] in PSUM.  The PSUM drain is then a
    bag-agnostic relu at FD=1024 (alternating ScalarE / VectorE) into a bf16
    h^T in SBUF - no per-bag accum pieces on the slow engines.
  * The ragged per-bag segment sum runs on TensorE: a {0,1} indicator matrix
    S[g] [128 inst, 32 bags] as stationary, h^T as moving, 4-way column
    tiling (tile_position), accumulating 4 PSUM band rows across all groups.
  * Final: one tensor_tensor_reduce contracts the pooled bands with W2 along
    the free axis, a [128,32] fold matmul adds the 4 bands per bag, then
    scale by 1/n_b and add b2 in a single tensor_scalar.
  * PE warmup: dummy matmuls during the initial DMA window so the HAM clock
    gate reaches 2.4 GHz before the real matmul stream starts.

b1 general-path note: padded columns are zeros and excluded by S, so no
relu(b1) correction is ever needed.  A nonzero b1 is folded in via an extra
K=1 matmul per group (ones-row x b1-row); the spec's b1 is all-zero so the
compiled program skips it.
"""
import os
import sys
import numpy as np

sys.path.insert(0, '/opt/trn_rl_repo')

# debug knobs (default = full-featured kernel)
DBG_NOWARM = os.environ.get('KDBG_NOWARM', '0') == '1'
DBG_NOTILEPOS = os.environ.get('KDBG_NOTILEPOS', '0') == '1'
DBG_NOPASS2 = os.environ.get('KDBG_NOPASS2', '0') == '1'
DBG_ALLSCALAR = os.environ.get('KDBG_ALLSCALAR', '0') == '1'
DBG_NOFINAL = os.environ.get('KDBG_NOFINAL', '0') == '1'
DBG_NODRAIN = os.environ.get('KDBG_NODRAIN', '0') == '1'

B, N_MAX, D_IN, D_H = 256, 512, 256, 256
N_CORES = 8
BAGS = B // N_CORES          # 32 bag slots per core
GPB = 4                      # instance groups per PSUM buffer
N_WARM = 6                   # warmup matmuls to bridge DMA wait

_PROGRAMS = {}


def _build_program(G, b1_nonzero):
    import concourse.bacc as bacc
    import concourse.tile as tile
    from concourse import mybir

    f32 = mybir.dt.float32
    bf16 = mybir.dt.bfloat16
    Alu = mybir.AluOpType
    Act = mybir.ActivationFunctionType

    W = G * 128
    NBLK = G // GPB

    nc = bacc.Bacc("TRN2", target_bir_lowering=False, debug=False)

    xt = nc.dram_tensor("xt", [D_IN, W], bf16, kind="ExternalInput").ap()
    w1 = nc.dram_tensor("w1", [D_IN, D_H], bf16, kind="ExternalInput").ap()
    s_t = nc.dram_tensor("s_t", [128, G * BAGS], bf16, kind="ExternalInput").ap()
    w2b = nc.dram_tensor("w2b", [128, D_H], f32, kind="ExternalInput").ap()
    fold = nc.dram_tensor("fold", [128, BAGS], f32, kind="ExternalInput").ap()
    inv = nc.dram_tensor("inv", [BAGS, 1], f32, kind="ExternalInput").ap()
    bias2 = nc.dram_tensor("bias2", [BAGS, 1], f32, kind="ExternalInput").ap()
    if b1_nonzero:
        b1r = nc.dram_tensor("b1r", [1, D_H], bf16, kind="ExternalInput").ap()
    out = nc.dram_tensor("out", [BAGS, 1], f32, kind="ExternalOutput").ap()

    with tile.TileContext(nc) as tc:
        with (
            tc.tile_pool(name="const", bufs=1) as cpool,
            tc.tile_pool(name="xsb", bufs=1) as xpool,
            tc.tile_pool(name="hsb", bufs=1) as hpool,
            tc.tile_pool(name="z", bufs=3, space="PSUM") as zpool,
            tc.tile_pool(name="sps", bufs=1, space="PSUM") as spool,
        ):
            # ---- PE warmup: zeros matmuls fill the HAM activity window ----
            warm = cpool.tile([128, 256], bf16, tag="warm")
            nc.vector.memset(warm[:], 0.0)
            small = spool.tile([128, 512], f32, tag="small")
            bands = small[:, 0:D_H]            # 4 x 32 bag-band rows
            pot = spool.tile([BAGS, 1], f32, tag="pot")
            po = pot[:]
            for _ in range(0 if DBG_NOWARM else N_WARM):
                nc.tensor.matmul(small[:, 256:512], warm[:, 0:128], warm[:],
                                 start=True, stop=True, skip_group_check=True)

            # ---- constants + x prefetch (HWDGE, bf16 host-cast) ----
            w1k0 = cpool.tile([128, D_H], bf16, tag="w1k0")
            w1k1 = cpool.tile([128, D_H], bf16, tag="w1k1")
            nc.sync.dma_start(w1k0[:], w1[0:128, :])
            nc.sync.dma_start(w1k1[:], w1[128:256, :])
            xk0 = xpool.tile([128, W], bf16, tag="xk0")
            xk1 = xpool.tile([128, W], bf16, tag="xk1")
            gb = [0, 6 * 128, 18 * 128, 36 * 128, W]
            bounds = sorted(set(min(b, W) for b in gb))
            for ci in range(len(bounds) - 1):
                sl = slice(bounds[ci], bounds[ci + 1])
                if sl.start >= sl.stop:
                    continue
                nc.sync.dma_start(xk0[:, sl], xt[0:128, sl])
                nc.gpsimd.dma_start(xk1[:, sl], xt[128:256, sl])
            s_sb = cpool.tile([128, G * BAGS], bf16, tag="s_sb")
            nc.scalar.dma_start(s_sb[:], s_t[:])
            w2sb = cpool.tile([128, D_H], f32, tag="w2sb")
            nc.scalar.dma_start(w2sb[:], w2b[:])
            foldsb = cpool.tile([128, BAGS], f32, tag="foldsb")
            nc.scalar.dma_start(foldsb[:], fold[:])
            invsb = cpool.tile([BAGS, 1], f32, tag="invsb")
            nc.scalar.dma_start(invsb[:], inv[:])
            b2sb = cpool.tile([BAGS, 1], f32, tag="b2sb")
            nc.scalar.dma_start(b2sb[:], bias2[:])
            if b1_nonzero:
                onesr = cpool.tile([1, 128], bf16, tag="onesr")
                nc.vector.memset(onesr[:], 1.0)
                b1sb = cpool.tile([1, D_H], bf16, tag="b1sb")
                nc.sync.dma_start(b1sb[:], b1r[:])

            hT = hpool.tile([128, G * D_H], bf16, tag="hT")

            # ---- main loop: GPB instance groups per PSUM buffer ----
            for blk in range(NBLK):
                zb = zpool.tile([128, GPB * D_H], f32, tag="zb")
                for j in range(GPB):
                    g = GPB * blk + j
                    zsl = zb[:, D_H * j:D_H * (j + 1)]
                    xsl = slice(128 * g, 128 * (g + 1))
                    nc.tensor.matmul(zsl, xk0[:, xsl], w1k0[:],
                                     start=True, stop=False)
                    nc.tensor.matmul(zsl, xk1[:, xsl], w1k1[:],
                                     start=False, stop=not b1_nonzero)
                    if b1_nonzero:
                        nc.tensor.matmul(zsl, onesr[:], b1sb[:],
                                         start=False, stop=True)
                # relu drain, whole buffer in one big-FD instruction
                hsl = hT[:, GPB * D_H * blk:GPB * D_H * (blk + 1)]
                if DBG_NODRAIN:
                    pass
                elif DBG_ALLSCALAR or blk % 2 == 1:
                    nc.scalar.activation(hsl, zb[:], Act.Relu,
                                         bias=0.0, scale=1.0)
                else:
                    nc.vector.tensor_scalar(hsl, zb[:], 0.0, None, op0=Alu.max)
                # segment-sum matmuls: indicator stationary, 4-way col tiling
                if not DBG_NOPASS2:
                    for j in range(GPB):
                        g = GPB * blk + j
                        jt = j % 4
                        kw = {}
                        if not DBG_NOTILEPOS:
                            kw['tile_position'] = (0, 32 * jt)
                        nc.tensor.matmul(
                            bands[32 * jt:32 * (jt + 1), :],
                            s_sb[:, BAGS * g:BAGS * (g + 1)],
                            hT[:, D_H * g:D_H * (g + 1)],
                            start=(g < 4), stop=(g >= G - 4),
                            skip_group_check=True, **kw)

            # ---- W2 contraction + band fold + mean + bias ----
            scr = cpool.tile([128, D_H], f32, tag="scr")
            acc = cpool.tile([128, 1], f32, tag="acc")
            osb = cpool.tile([BAGS, 1], f32, tag="osb")
            if DBG_NOFINAL:
                nc.vector.memset(osb[:], 0.0)
            else:
                # acc[p] = sum_dh bands[p, dh] * W2[dh]  (proven stt+accum form)
                nc.vector.scalar_tensor_tensor(
                    scr[:], bands, 0.0, w2sb[:], op0=Alu.add, op1=Alu.mult,
                    accum_out=acc[:])
                nc.tensor.matmul(po, foldsb[:], acc[:], start=True, stop=True,
                                 skip_group_check=True)
                nc.vector.tensor_scalar(osb[:], po, invsb[:, 0:1], None,
                                        op0=Alu.mult)
                nc.vector.tensor_add(osb[:], osb[:], b2sb[:])
            nc.sync.dma_start(out[:], osb[:])

    nc.compile()
    return nc


def get_program(G, b1_nonzero):
    key = (int(G), bool(b1_nonzero))
    if key not in _PROGRAMS:
        _PROGRAMS[key] = _build_program(*key)
    return _PROGRAMS[key]


def _plan(n):
    """Snake-deal bags (sorted by size, desc) across cores; return
    assignment[core][slot] -> bag id and G (shared group count)."""
    order = np.argsort(-n, kind='stable')
    assignment = np.empty((N_CORES, BAGS), dtype=np.int64)
    for i, bag in enumerate(order):
        r, p = divmod(i, N_CORES)
        core = p if (r % 2 == 0) else (N_CORES - 1 - p)
        assignment[core, r] = bag
    v_max = max(int(n[assignment[c]].sum()) for c in range(N_CORES))
    G = -(-v_max // 128)
    G = -(-G // GPB) * GPB          # multiple of GPB (psum buffer / col tiles)
    return assignment, G


def make_in_maps(x, n_instances, W1, b1, W2, b2=None):
    import ml_dtypes
    bf16 = ml_dtypes.bfloat16

    x = np.asarray(x, dtype=np.float32)
    n = np.asarray(n_instances, dtype=np.int64)
    W1 = np.asarray(W1, dtype=np.float32)
    b1 = np.asarray(b1, dtype=np.float32).reshape(-1)
    W2 = np.asarray(W2, dtype=np.float32).reshape(-1)
    b2v = float(np.asarray(b2).reshape(-1)[0]) if b2 is not None else 0.0

    assignment, G = _plan(n)
    W = G * 128
    b1_nonzero = bool(np.any(b1 != 0.0))

    xflat = x.reshape(B * N_MAX, D_IN)
    w1_bf = np.ascontiguousarray(W1.astype(bf16))
    w2b = np.ascontiguousarray(
        np.broadcast_to(W2.reshape(1, D_H), (128, D_H)).astype(np.float32))
    foldm = np.zeros((128, BAGS), dtype=np.float32)
    foldm[np.arange(128), np.arange(128) % BAGS] = 1.0

    in_maps = []
    for c in range(N_CORES):
        bags = assignment[c]
        ns = n[bags]
        v = int(ns.sum())
        # gather valid instance rows: bag-major, instance-minor
        idx = np.concatenate(
            [bags[s] * N_MAX + np.arange(ns[s]) for s in range(BAGS)])
        xt = np.zeros((D_IN, W), dtype=bf16)
        xt[:, :v] = xflat[idx].T.astype(bf16)
        # indicator S: [W, 32] -> [128, G*32]
        starts = np.zeros(BAGS + 1, dtype=np.int64)
        np.cumsum(ns, out=starts[1:])
        s_full = np.zeros((W, BAGS), dtype=bf16)
        for s in range(BAGS):
            s_full[starts[s]:starts[s + 1], s] = bf16(1.0)
        s_t = np.ascontiguousarray(
            s_full.reshape(G, 128, BAGS).transpose(1, 0, 2).reshape(128, G * BAGS))
        im = {
            "xt": xt,
            "w1": w1_bf,
            "s_t": s_t,
            "w2b": w2b,
            "fold": foldm,
            "inv": (1.0 / ns.astype(np.float32)).reshape(BAGS, 1),
            "bias2": np.full((BAGS, 1), b2v, dtype=np.float32),
        }
        if b1_nonzero:
            im["b1r"] = np.ascontiguousarray(b1.reshape(1, D_H).astype(bf16))
        in_maps.append(im)
    return in_maps


def run_spmd(in_maps, b2_value=0.0, trace=False, **kwargs):
    from concourse import bass_utils
    if trace:
        # no S3 in this environment; keep trace artifacts local
        bass_utils.upload_artifacts = lambda tmpdir: tmpdir
    G = in_maps[0]["xt"].shape[1] // 128
    nc = get_program(G, "b1r" in in_maps[0])
    return bass_utils.run_bass_kernel_spmd(
        nc, in_maps, core_ids=list(range(N_CORES)), trace=trace, **kwargs)


def kernel(x, n_instances, W1, b1, W2, b2):
    n = np.asarray(n_instances, dtype=np.int64)
    assignment, _ = _plan(n)
    in_maps = make_in_maps(x, n_instances, W1, b1, W2, b2)
    res = run_spmd(in_maps)
    out = np.empty((B, 1), dtype=np.float32)
    for c in range(N_CORES):
        out[assignment[c]] = np.asarray(res.results[c]["out"],
                                        dtype=np.float32).reshape(BAGS, 1)
    return out


# revision 13
# speedup vs baseline: 1.1977x; 1.0983x over previous
"""Trainium2 Bass kernel for nn_BagModel_3d (segment_reduce).

Computation (per bag b):
  out[b] = (1/n_b) * sum_{i < n_b} relu(x[b, i, :] @ W1 + b1) @ W2 + b2

Strategy (8 cores, data-parallel over bags):
  * Host: sort bags by n_instances, snake-deal across cores (balanced work),
    concatenate ONLY the valid instances per core (exact compaction - the
    random n_b average ~256/512, so this halves DMA and matmul work), cast
    to bf16, zero-pad to G*128 columns.
  * Device, instance-major layout: for each 128-instance group, the x tile
    [d_in=128, inst=128] is the matmul STATIONARY operand and W1 [128, 256]
    the moving one, giving z^T [inst, dh] in PSUM.  The PSUM drain is then a
    bag-agnostic relu at FD=1024 (alternating ScalarE / VectorE) into a bf16
    h^T in SBUF - no per-bag accum pieces on the slow engines.
  * The ragged per-bag segment sum runs on TensorE: a {0,1} indicator matrix
    S[g] [128 inst, 32 bags] as stationary, h^T as moving, 4-way column
    tiling (tile_position), accumulating 4 PSUM band rows across all groups.
  * Final: one tensor_tensor_reduce contracts the pooled bands with W2 along
    the free axis, a [128,32] fold matmul adds the 4 bands per bag, then
    scale by 1/n_b and add b2 in a single tensor_scalar.
  * PE warmup: dummy matmuls during the initial DMA window so the HAM clock
    gate reaches 2.4 GHz before the real matmul stream starts.

b1 general-path note: padded columns are zeros and excluded by S, so no
relu(b1) correction is ever needed.  A nonzero b1 is folded in via an extra
K=1 matmul per group (ones-row x b1-row); the spec's b1 is all-zero so the
compiled program skips it.
"""
import os
import sys
import numpy as np

sys.path.insert(0, '/opt/trn_rl_repo')

# debug knobs (default = full-featured kernel)
DBG_NOWARM = os.environ.get('KDBG_NOWARM', '0') == '1'
DBG_NOTILEPOS = os.environ.get('KDBG_NOTILEPOS', '0') == '1'
DBG_NOPASS2 = os.environ.get('KDBG_NOPASS2', '0') == '1'
DBG_ALLSCALAR = os.environ.get('KDBG_ALLSCALAR', '0') == '1'
DBG_NOFINAL = os.environ.get('KDBG_NOFINAL', '0') == '1'
DBG_NODRAIN = os.environ.get('KDBG_NODRAIN', '0') == '1'

B, N_MAX, D_IN, D_H = 256, 512, 256, 256
N_CORES = 8
BAGS = B // N_CORES          # 32 bag slots per core
GPB = 4                      # instance groups per PSUM buffer
N_WARM = 10                  # warmup matmuls to bridge DMA wait
NB16 = 2                     # tail blocks kept in bf16 (smallest bags)

_PROGRAMS = {}


def _build_program(G, G8, b1_nonzero):
    import concourse.bacc as bacc
    import concourse.tile as tile
    from concourse import mybir

    f32 = mybir.dt.float32
    bf16 = mybir.dt.bfloat16
    fp8 = mybir.dt.float8e4
    Alu = mybir.AluOpType
    Act = mybir.ActivationFunctionType

    W = G * 128
    W8 = G8 * 128
    W16 = W - W8
    NBLK = G // GPB

    nc = bacc.Bacc("TRN2", target_bir_lowering=False, debug=False)

    xt = nc.dram_tensor("xt", [D_IN, W8], fp8, kind="ExternalInput").ap()
    xt16 = (nc.dram_tensor("xt16", [D_IN, W16], bf16, kind="ExternalInput").ap()
            if W16 else None)
    w1 = nc.dram_tensor("w1", [128, 2 * D_H], bf16, kind="ExternalInput").ap()
    s_t = nc.dram_tensor("s_t", [128, G * BAGS], bf16, kind="ExternalInput").ap()
    w2b = nc.dram_tensor("w2b", [128, D_H], f32, kind="ExternalInput").ap()
    fold = nc.dram_tensor("fold", [128, BAGS], f32, kind="ExternalInput").ap()
    inv = nc.dram_tensor("inv", [BAGS, 1], f32, kind="ExternalInput").ap()
    bias2 = nc.dram_tensor("bias2", [BAGS, 1], f32, kind="ExternalInput").ap()
    if b1_nonzero:
        b1r = nc.dram_tensor("b1r", [1, D_H], bf16, kind="ExternalInput").ap()
    out = nc.dram_tensor("out", [BAGS, 1], f32, kind="ExternalOutput").ap()

    with tile.TileContext(nc) as tc:
        with (
            tc.tile_pool(name="const", bufs=1) as cpool,
            tc.tile_pool(name="xsb", bufs=1) as xpool,
            tc.tile_pool(name="hsb", bufs=1) as hpool,
            tc.tile_pool(name="z", bufs=3, space="PSUM") as zpool,
            tc.tile_pool(name="sps", bufs=1, space="PSUM") as spool,
        ):
            # ---- PE warmup: zeros matmuls fill the HAM activity window ----
            warm = cpool.tile([128, 256], bf16, tag="warm")
            nc.vector.memset(warm[:], 0.0)
            small = spool.tile([128, 512], f32, tag="small")
            bands = small[:, 0:D_H]            # 4 x 32 bag-band rows
            pot = spool.tile([BAGS, 1], f32, tag="pot")
            po = pot[:]
            for _ in range(0 if DBG_NOWARM else N_WARM):
                nc.tensor.matmul(small[:, 256:512], warm[:, 0:128], warm[:],
                                 start=True, stop=True, skip_group_check=True)

            # ---- constants + x prefetch (HWDGE, bf16 host-cast) ----
            w1cat = cpool.tile([128, 2 * D_H], bf16, tag="w1cat")
            nc.sync.dma_start(w1cat[:], w1[:])
            w1k0 = w1cat[:, 0:D_H]
            w1k1 = w1cat[:, D_H:2 * D_H]
            xk0 = xpool.tile([128, max(W8, 128)], fp8, tag="xk0")
            xk1 = xpool.tile([128, max(W8, 128)], fp8, tag="xk1")
            gb = [0, 2 * 128, 8 * 128, 20 * 128, 40 * 128, W8]
            bounds = sorted(set(min(b, W8) for b in gb))
            for ci in range(len(bounds) - 1):
                sl = slice(bounds[ci], bounds[ci + 1])
                if sl.start >= sl.stop:
                    continue
                nc.sync.dma_start(xk0[:, sl], xt[0:128, sl])
                eng = nc.scalar if ci >= 2 else nc.sync
                eng.dma_start(xk1[:, sl], xt[128:256, sl])
            if W16:
                xk16_0 = xpool.tile([128, W16], bf16, tag="xk16_0")
                xk16_1 = xpool.tile([128, W16], bf16, tag="xk16_1")
                nc.scalar.dma_start(xk16_0[:], xt16[0:128, :])
                nc.scalar.dma_start(xk16_1[:], xt16[128:256, :])
            s_sb = cpool.tile([128, G * BAGS], bf16, tag="s_sb")
            nc.scalar.dma_start(s_sb[:], s_t[:])
            w2sb = cpool.tile([128, D_H], f32, tag="w2sb")
            nc.scalar.dma_start(w2sb[:], w2b[:])
            foldsb = cpool.tile([128, BAGS], f32, tag="foldsb")
            nc.scalar.dma_start(foldsb[:], fold[:])
            invsb = cpool.tile([BAGS, 1], f32, tag="invsb")
            nc.scalar.dma_start(invsb[:], inv[:])
            b2sb = cpool.tile([BAGS, 1], f32, tag="b2sb")
            nc.scalar.dma_start(b2sb[:], bias2[:])
            if b1_nonzero:
                onesr = cpool.tile([1, 128], bf16, tag="onesr")
                nc.vector.memset(onesr[:], 16.0)
                onesr1 = cpool.tile([1, 128], bf16, tag="onesr1")
                nc.vector.memset(onesr1[:], 1.0)
                b1sb = cpool.tile([1, D_H], bf16, tag="b1sb")
                nc.sync.dma_start(b1sb[:], b1r[:])

            hT = hpool.tile([128, G * D_H], bf16, tag="hT")

            # ---- main loop: GPB instance groups per PSUM buffer ----
            for blk in range(NBLK):
                zb = zpool.tile([128, GPB * D_H], f32, tag="zb")
                is8 = blk < G8 // GPB
                for j in range(GPB):
                    g = GPB * blk + j
                    zsl = zb[:, D_H * j:D_H * (j + 1)]
                    if is8:
                        xsl = slice(128 * g, 128 * (g + 1))
                        a0, a1 = xk0[:, xsl], xk1[:, xsl]
                    else:
                        xsl = slice(128 * (g - G8), 128 * (g - G8 + 1))
                        a0, a1 = xk16_0[:, xsl], xk16_1[:, xsl]
                    nc.tensor.matmul(zsl, a0, w1k0,
                                     start=True, stop=False)
                    nc.tensor.matmul(zsl, a1, w1k1,
                                     start=False, stop=not b1_nonzero)
                    if b1_nonzero:
                        nc.tensor.matmul(zsl, onesr[:] if is8 else onesr1[:],
                                         b1sb[:], start=False, stop=True)
                # relu drain, whole buffer in one big-FD instruction
                hsl = hT[:, GPB * D_H * blk:GPB * D_H * (blk + 1)]
                if DBG_NODRAIN:
                    pass
                elif DBG_ALLSCALAR or blk % 2 == 1:
                    nc.scalar.activation(hsl, zb[:], Act.Relu,
                                         bias=0.0, scale=(1.0 / 16.0) if is8 else 1.0)
                else:
                    nc.vector.tensor_scalar(hsl, zb[:], (1.0 / 16.0) if is8 else 1.0,
                                            0.0, op0=Alu.mult, op1=Alu.max)
                # segment-sum matmuls: indicator stationary, 4-way col tiling
                if not DBG_NOPASS2:
                    for j in range(GPB):
                        g = GPB * blk + j
                        jt = j % 4
                        kw = {}
                        if not DBG_NOTILEPOS:
                            kw['tile_position'] = (0, 32 * jt)
                        nc.tensor.matmul(
                            bands[32 * jt:32 * (jt + 1), :],
                            s_sb[:, BAGS * g:BAGS * (g + 1)],
                            hT[:, D_H * g:D_H * (g + 1)],
                            start=(g < 4), stop=(g >= G - 4),
                            skip_group_check=True, **kw)

            # ---- W2 contraction + band fold + mean + bias ----
            scr = cpool.tile([128, D_H], f32, tag="scr")
            acc = cpool.tile([128, 1], f32, tag="acc")
            osb = cpool.tile([BAGS, 1], f32, tag="osb")
            if DBG_NOFINAL:
                nc.vector.memset(osb[:], 0.0)
            else:
                # acc[p] = sum_dh bands[p, dh] * W2[dh]  (proven stt+accum form)
                nc.vector.scalar_tensor_tensor(
                    scr[:], bands, 0.0, w2sb[:], op0=Alu.add, op1=Alu.mult,
                    accum_out=acc[:])
                nc.tensor.matmul(po, foldsb[:], acc[:], start=True, stop=True,
                                 skip_group_check=True)
                nc.vector.tensor_scalar(osb[:], po, invsb[:, 0:1], None,
                                        op0=Alu.mult)
                nc.vector.tensor_add(osb[:], osb[:], b2sb[:])
            nc.sync.dma_start(out[:], osb[:])

    nc.compile()
    return nc


def get_program(G, G8, b1_nonzero):
    key = (int(G), int(G8), bool(b1_nonzero))
    if key not in _PROGRAMS:
        _PROGRAMS[key] = _build_program(*key)
    return _PROGRAMS[key]


def _plan(n):
    """Snake-deal bags (sorted by size, desc) across cores; return
    assignment[core][slot] -> bag id and G (shared group count)."""
    order = np.argsort(-n, kind='stable')
    assignment = np.empty((N_CORES, BAGS), dtype=np.int64)
    for i, bag in enumerate(order):
        r, p = divmod(i, N_CORES)
        core = p if (r % 2 == 0) else (N_CORES - 1 - p)
        assignment[core, r] = bag
    v_max = max(int(n[assignment[c]].sum()) for c in range(N_CORES))
    G = -(-v_max // 128)
    G = -(-G // GPB) * GPB          # multiple of GPB (psum buffer / col tiles)
    G8 = max(G - NB16 * GPB, 0)     # leading fp8 region; bf16 tail
    return assignment, G, G8


def make_in_maps(x, n_instances, W1, b1, W2, b2=None):
    import ml_dtypes
    bf16 = ml_dtypes.bfloat16
    fp8 = ml_dtypes.float8_e4m3

    x = np.asarray(x, dtype=np.float32)
    n = np.asarray(n_instances, dtype=np.int64)
    W1 = np.asarray(W1, dtype=np.float32)
    b1 = np.asarray(b1, dtype=np.float32).reshape(-1)
    W2 = np.asarray(W2, dtype=np.float32).reshape(-1)
    b2v = float(np.asarray(b2).reshape(-1)[0]) if b2 is not None else 0.0

    assignment, G, G8 = _plan(n)
    W = G * 128
    W8 = G8 * 128
    b1_nonzero = bool(np.any(b1 != 0.0))

    xflat = x.reshape(B * N_MAX, D_IN)
    w1_bf = np.ascontiguousarray(
        np.concatenate([W1[0:128, :], W1[128:256, :]], axis=1).astype(bf16))
    w2b = np.ascontiguousarray(
        np.broadcast_to(W2.reshape(1, D_H), (128, D_H)).astype(np.float32))
    foldm = np.zeros((128, BAGS), dtype=np.float32)
    foldm[np.arange(128), np.arange(128) % BAGS] = 1.0

    in_maps = []
    for c in range(N_CORES):
        bags = assignment[c]
        ns = n[bags]
        v = int(ns.sum())
        # gather valid instance rows: bag-major, instance-minor
        idx = np.concatenate(
            [bags[s] * N_MAX + np.arange(ns[s]) for s in range(BAGS)])
        xfull = np.zeros((D_IN, W), dtype=np.float32)
        xfull[:, :v] = xflat[idx].T
        xt = (xfull[:, :W8] * np.float32(16.0)).astype(fp8)
        xt16 = xfull[:, W8:].astype(bf16)
        # indicator S: [W, 32] -> [128, G*32]
        starts = np.zeros(BAGS + 1, dtype=np.int64)
        np.cumsum(ns, out=starts[1:])
        s_full = np.zeros((W, BAGS), dtype=bf16)
        for s in range(BAGS):
            s_full[starts[s]:starts[s + 1], s] = bf16(1.0)
        s_t = np.ascontiguousarray(
            s_full.reshape(G, 128, BAGS).transpose(1, 0, 2).reshape(128, G * BAGS))
        im = {
            "xt": xt,
            "xt16": xt16,
            "w1": w1_bf,
            "s_t": s_t,
            "w2b": w2b,
            "fold": foldm,
            "inv": (1.0 / ns.astype(np.float32)).reshape(BAGS, 1),
            "bias2": np.full((BAGS, 1), b2v, dtype=np.float32),
        }
        if b1_nonzero:
            im["b1r"] = np.ascontiguousarray(b1.reshape(1, D_H).astype(bf16))
        in_maps.append(im)
    return in_maps


def run_spmd(in_maps, b2_value=0.0, trace=False, **kwargs):
    from concourse import bass_utils
    if trace:
        # no S3 in this environment; keep trace artifacts local
        bass_utils.upload_artifacts = lambda tmpdir: tmpdir
    G8 = in_maps[0]["xt"].shape[1] // 128
    G = G8 + in_maps[0]["xt16"].shape[1] // 128
    nc = get_program(G, G8, "b1r" in in_maps[0])
    return bass_utils.run_bass_kernel_spmd(
        nc, in_maps, core_ids=list(range(N_CORES)), trace=trace, **kwargs)


def kernel(x, n_instances, W1, b1, W2, b2):
    n = np.asarray(n_instances, dtype=np.int64)
    assignment = _plan(n)[0]
    in_maps = make_in_maps(x, n_instances, W1, b1, W2, b2)
    res = run_spmd(in_maps)
    out = np.empty((B, 1), dtype=np.float32)
    for c in range(N_CORES):
        out[assignment[c]] = np.asarray(res.results[c]["out"],
                                        dtype=np.float32).reshape(BAGS, 1)
    return out


# revision 14
# speedup vs baseline: 1.2244x; 1.0223x over previous
"""Trainium2 Bass kernel for nn_BagModel_3d (segment_reduce).

Computation (per bag b):
  out[b] = (1/n_b) * sum_{i < n_b} relu(x[b, i, :] @ W1 + b1) @ W2 + b2

Strategy (8 cores, data-parallel over bags):
  * Host: sort bags by n_instances, snake-deal across cores (balanced work),
    concatenate ONLY the valid instances per core (exact compaction - the
    random n_b average ~256/512, so this halves DMA and matmul work), cast
    to bf16, zero-pad to G*128 columns.
  * Device, instance-major layout: for each 128-instance group, the x tile
    [d_in=128, inst=128] is the matmul STATIONARY operand and W1 [128, 256]
    the moving one, giving z^T [inst, dh] in PSUM.  The PSUM drain is then a
    bag-agnostic relu at FD=1024 (alternating ScalarE / VectorE) into a bf16
    h^T in SBUF - no per-bag accum pieces on the slow engines.
  * The ragged per-bag segment sum runs on TensorE: a {0,1} indicator matrix
    S[g] [128 inst, 32 bags] as stationary, h^T as moving, 4-way column
    tiling (tile_position), accumulating 4 PSUM band rows across all groups.
  * Final: one tensor_tensor_reduce contracts the pooled bands with W2 along
    the free axis, a [128,32] fold matmul adds the 4 bands per bag, then
    scale by 1/n_b and add b2 in a single tensor_scalar.
  * PE warmup: dummy matmuls during the initial DMA window so the HAM clock
    gate reaches 2.4 GHz before the real matmul stream starts.

b1 general-path note: padded columns are zeros and excluded by S, so no
relu(b1) correction is ever needed.  A nonzero b1 is folded in via an extra
K=1 matmul per group (ones-row x b1-row); the spec's b1 is all-zero so the
compiled program skips it.
"""
import os
import sys
import numpy as np

sys.path.insert(0, '/opt/trn_rl_repo')

# debug knobs (default = full-featured kernel)
DBG_NOWARM = os.environ.get('KDBG_NOWARM', '0') == '1'
DBG_NOTILEPOS = os.environ.get('KDBG_NOTILEPOS', '0') == '1'
DBG_NOPASS2 = os.environ.get('KDBG_NOPASS2', '0') == '1'
DBG_ALLSCALAR = os.environ.get('KDBG_ALLSCALAR', '0') == '1'
DBG_NOFINAL = os.environ.get('KDBG_NOFINAL', '0') == '1'
DBG_NODRAIN = os.environ.get('KDBG_NODRAIN', '0') == '1'

B, N_MAX, D_IN, D_H = 256, 512, 256, 256
N_CORES = 8
BAGS = B // N_CORES          # 32 bag slots per core
GPB = 4                      # instance groups per PSUM buffer
N_WARM = 10                  # warmup matmuls to bridge DMA wait
NB16 = 2                     # tail blocks kept in bf16 (smallest bags)

_PROGRAMS = {}


def _build_program(G, G8, b1_nonzero):
    import concourse.bacc as bacc
    import concourse.tile as tile
    from concourse import mybir

    f32 = mybir.dt.float32
    bf16 = mybir.dt.bfloat16
    fp8 = mybir.dt.float8e4
    Alu = mybir.AluOpType
    Act = mybir.ActivationFunctionType

    W = G * 128
    W8 = G8 * 128
    W16 = W - W8
    NBLK = G // GPB

    nc = bacc.Bacc("TRN2", target_bir_lowering=False, debug=False)

    xt = nc.dram_tensor("xt", [D_IN, W8], fp8, kind="ExternalInput").ap()
    xt16 = (nc.dram_tensor("xt16", [D_IN, W16], bf16, kind="ExternalInput").ap()
            if W16 else None)
    w1 = nc.dram_tensor("w1", [128, 2 * D_H], bf16, kind="ExternalInput").ap()
    s_t = nc.dram_tensor("s_t", [128, G * BAGS], bf16, kind="ExternalInput").ap()
    w2b = nc.dram_tensor("w2b", [128, D_H], f32, kind="ExternalInput").ap()
    fold = nc.dram_tensor("fold", [128, BAGS], f32, kind="ExternalInput").ap()
    inv = nc.dram_tensor("inv", [BAGS, 1], f32, kind="ExternalInput").ap()
    bias2 = nc.dram_tensor("bias2", [BAGS, 1], f32, kind="ExternalInput").ap()
    if b1_nonzero:
        b1r = nc.dram_tensor("b1r", [1, D_H], bf16, kind="ExternalInput").ap()
    out = nc.dram_tensor("out", [BAGS, 1], f32, kind="ExternalOutput").ap()

    with tile.TileContext(nc) as tc:
        with (
            tc.tile_pool(name="const", bufs=1) as cpool,
            tc.tile_pool(name="xsb", bufs=1) as xpool,
            tc.tile_pool(name="hsb", bufs=1) as hpool,
            tc.tile_pool(name="z", bufs=3, space="PSUM") as zpool,
            tc.tile_pool(name="sps", bufs=1, space="PSUM") as spool,
        ):
            # ---- PE warmup: zeros matmuls fill the HAM activity window ----
            warm = cpool.tile([128, 256], bf16, tag="warm")
            nc.vector.memset(warm[:], 0.0)
            small = spool.tile([128, 512], f32, tag="small")
            bands = small[:, 0:D_H]            # 4 x 32 bag-band rows
            pot = spool.tile([BAGS, 1], f32, tag="pot")
            po = pot[:]
            for _ in range(0 if DBG_NOWARM else N_WARM):
                nc.tensor.matmul(small[:, 256:512], warm[:, 0:128], warm[:],
                                 start=True, stop=True, skip_group_check=True)

            # ---- constants + x prefetch (HWDGE, bf16 host-cast) ----
            w1cat = cpool.tile([128, 2 * D_H], bf16, tag="w1cat")
            nc.sync.dma_start(w1cat[:], w1[:])
            w1k0 = w1cat[:, 0:D_H]
            w1k1 = w1cat[:, D_H:2 * D_H]
            xk0 = xpool.tile([128, max(W8, 128)], fp8, tag="xk0")
            xk1 = xpool.tile([128, max(W8, 128)], fp8, tag="xk1")
            gb = [0, 2 * 128, 6 * 128, 12 * 128, 20 * 128, 30 * 128,
                  42 * 128, W8]
            bounds = sorted(set(min(b, W8) for b in gb))
            for ci in range(len(bounds) - 1):
                sl = slice(bounds[ci], bounds[ci + 1])
                if sl.start >= sl.stop:
                    continue
                nc.sync.dma_start(xk0[:, sl], xt[0:128, sl])
                nc.scalar.dma_start(xk1[:, sl], xt[128:256, sl])
            if W16:
                xk16_0 = xpool.tile([128, W16], bf16, tag="xk16_0")
                xk16_1 = xpool.tile([128, W16], bf16, tag="xk16_1")
                nc.scalar.dma_start(xk16_0[:], xt16[0:128, :])
                nc.scalar.dma_start(xk16_1[:], xt16[128:256, :])
            s_sb = cpool.tile([128, G * BAGS], bf16, tag="s_sb")
            nc.scalar.dma_start(s_sb[:], s_t[:])
            w2sb = cpool.tile([128, D_H], f32, tag="w2sb")
            nc.scalar.dma_start(w2sb[:], w2b[:])
            foldsb = cpool.tile([128, BAGS], f32, tag="foldsb")
            nc.scalar.dma_start(foldsb[:], fold[:])
            invsb = cpool.tile([BAGS, 1], f32, tag="invsb")
            nc.scalar.dma_start(invsb[:], inv[:])
            b2sb = cpool.tile([BAGS, 1], f32, tag="b2sb")
            nc.scalar.dma_start(b2sb[:], bias2[:])
            if b1_nonzero:
                onesr = cpool.tile([1, 128], bf16, tag="onesr")
                nc.vector.memset(onesr[:], 16.0)
                onesr1 = cpool.tile([1, 128], bf16, tag="onesr1")
                nc.vector.memset(onesr1[:], 1.0)
                b1sb = cpool.tile([1, D_H], bf16, tag="b1sb")
                nc.sync.dma_start(b1sb[:], b1r[:])

            hT = hpool.tile([128, G * D_H], bf16, tag="hT")

            # ---- main loop: GPB instance groups per PSUM buffer ----
            for blk in range(NBLK):
                zb = zpool.tile([128, GPB * D_H], f32, tag="zb")
                is8 = blk < G8 // GPB
                for j in range(GPB):
                    g = GPB * blk + j
                    zsl = zb[:, D_H * j:D_H * (j + 1)]
                    if is8:
                        xsl = slice(128 * g, 128 * (g + 1))
                        a0, a1 = xk0[:, xsl], xk1[:, xsl]
                    else:
                        xsl = slice(128 * (g - G8), 128 * (g - G8 + 1))
                        a0, a1 = xk16_0[:, xsl], xk16_1[:, xsl]
                    nc.tensor.matmul(zsl, a0, w1k0,
                                     start=True, stop=False)
                    nc.tensor.matmul(zsl, a1, w1k1,
                                     start=False, stop=not b1_nonzero)
                    if b1_nonzero:
                        nc.tensor.matmul(zsl, onesr[:] if is8 else onesr1[:],
                                         b1sb[:], start=False, stop=True)
                # relu drain, whole buffer in one big-FD instruction
                hsl = hT[:, GPB * D_H * blk:GPB * D_H * (blk + 1)]
                if DBG_NODRAIN:
                    pass
                elif DBG_ALLSCALAR or blk % 2 == 1:
                    nc.scalar.activation(hsl, zb[:], Act.Relu,
                                         bias=0.0, scale=(1.0 / 16.0) if is8 else 1.0)
                else:
                    nc.vector.tensor_scalar(hsl, zb[:], (1.0 / 16.0) if is8 else 1.0,
                                            0.0, op0=Alu.mult, op1=Alu.max)
                # segment-sum matmuls: indicator stationary, 4-way col tiling
                if not DBG_NOPASS2:
                    for j in range(GPB):
                        g = GPB * blk + j
                        jt = j % 4
                        kw = {}
                        if not DBG_NOTILEPOS:
                            kw['tile_position'] = (0, 32 * jt)
                        nc.tensor.matmul(
                            bands[32 * jt:32 * (jt + 1), :],
                            s_sb[:, BAGS * g:BAGS * (g + 1)],
                            hT[:, D_H * g:D_H * (g + 1)],
                            start=(g < 4), stop=(g >= G - 4),
                            skip_group_check=True, **kw)

            # ---- W2 contraction + band fold + mean + bias ----
            scr = cpool.tile([128, D_H], f32, tag="scr")
            acc = cpool.tile([128, 1], f32, tag="acc")
            osb = cpool.tile([BAGS, 1], f32, tag="osb")
            if DBG_NOFINAL:
                nc.vector.memset(osb[:], 0.0)
            else:
                # acc[p] = sum_dh bands[p, dh] * W2[dh]  (proven stt+accum form)
                nc.vector.scalar_tensor_tensor(
                    scr[:], bands, 0.0, w2sb[:], op0=Alu.add, op1=Alu.mult,
                    accum_out=acc[:])
                nc.tensor.matmul(po, foldsb[:], acc[:], start=True, stop=True,
                                 skip_group_check=True)
                nc.vector.tensor_scalar(osb[:], po, invsb[:, 0:1],
                                        b2sb[:, 0:1], op0=Alu.mult,
                                        op1=Alu.add)
            nc.sync.dma_start(out[:], osb[:])

    nc.compile()
    return nc


def get_program(G, G8, b1_nonzero):
    key = (int(G), int(G8), bool(b1_nonzero))
    if key not in _PROGRAMS:
        _PROGRAMS[key] = _build_program(*key)
    return _PROGRAMS[key]


def _plan(n):
    """Snake-deal bags (sorted by size, desc) across cores; return
    assignment[core][slot] -> bag id and G (shared group count)."""
    order = np.argsort(-n, kind='stable')
    assignment = np.empty((N_CORES, BAGS), dtype=np.int64)
    for i, bag in enumerate(order):
        r, p = divmod(i, N_CORES)
        core = p if (r % 2 == 0) else (N_CORES - 1 - p)
        assignment[core, r] = bag
    v_max = max(int(n[assignment[c]].sum()) for c in range(N_CORES))
    G = -(-v_max // 128)
    G = -(-G // GPB) * GPB          # multiple of GPB (psum buffer / col tiles)
    G8 = max(G - NB16 * GPB, 0)     # leading fp8 region; bf16 tail
    return assignment, G, G8


def make_in_maps(x, n_instances, W1, b1, W2, b2=None):
    import ml_dtypes
    bf16 = ml_dtypes.bfloat16
    fp8 = ml_dtypes.float8_e4m3

    x = np.asarray(x, dtype=np.float32)
    n = np.asarray(n_instances, dtype=np.int64)
    W1 = np.asarray(W1, dtype=np.float32)
    b1 = np.asarray(b1, dtype=np.float32).reshape(-1)
    W2 = np.asarray(W2, dtype=np.float32).reshape(-1)
    b2v = float(np.asarray(b2).reshape(-1)[0]) if b2 is not None else 0.0

    assignment, G, G8 = _plan(n)
    W = G * 128
    W8 = G8 * 128
    b1_nonzero = bool(np.any(b1 != 0.0))

    xflat = x.reshape(B * N_MAX, D_IN)
    w1_bf = np.ascontiguousarray(
        np.concatenate([W1[0:128, :], W1[128:256, :]], axis=1).astype(bf16))
    w2b = np.ascontiguousarray(
        np.broadcast_to(W2.reshape(1, D_H), (128, D_H)).astype(np.float32))
    foldm = np.zeros((128, BAGS), dtype=np.float32)
    foldm[np.arange(128), np.arange(128) % BAGS] = 1.0

    in_maps = []
    for c in range(N_CORES):
        bags = assignment[c]
        ns = n[bags]
        v = int(ns.sum())
        # gather valid instance rows: bag-major, instance-minor
        idx = np.concatenate(
            [bags[s] * N_MAX + np.arange(ns[s]) for s in range(BAGS)])
        xfull = np.zeros((D_IN, W), dtype=np.float32)
        xfull[:, :v] = xflat[idx].T
        xt = (xfull[:, :W8] * np.float32(16.0)).astype(fp8)
        xt16 = xfull[:, W8:].astype(bf16)
        # indicator S: [W, 32] -> [128, G*32]
        starts = np.zeros(BAGS + 1, dtype=np.int64)
        np.cumsum(ns, out=starts[1:])
        s_full = np.zeros((W, BAGS), dtype=bf16)
        for s in range(BAGS):
            s_full[starts[s]:starts[s + 1], s] = bf16(1.0)
        s_t = np.ascontiguousarray(
            s_full.reshape(G, 128, BAGS).transpose(1, 0, 2).reshape(128, G * BAGS))
        im = {
            "xt": xt,
            "xt16": xt16,
            "w1": w1_bf,
            "s_t": s_t,
            "w2b": w2b,
            "fold": foldm,
            "inv": (1.0 / ns.astype(np.float32)).reshape(BAGS, 1),
            "bias2": np.full((BAGS, 1), b2v, dtype=np.float32),
        }
        if b1_nonzero:
            im["b1r"] = np.ascontiguousarray(b1.reshape(1, D_H).astype(bf16))
        in_maps.append(im)
    return in_maps


def run_spmd(in_maps, b2_value=0.0, trace=False, **kwargs):
    from concourse import bass_utils
    if trace:
        # no S3 in this environment; keep trace artifacts local
        bass_utils.upload_artifacts = lambda tmpdir: tmpdir
    G8 = in_maps[0]["xt"].shape[1] // 128
    G = G8 + in_maps[0]["xt16"].shape[1] // 128
    nc = get_program(G, G8, "b1r" in in_maps[0])
    return bass_utils.run_bass_kernel_spmd(
        nc, in_maps, core_ids=list(range(N_CORES)), trace=trace, **kwargs)


def kernel(x, n_instances, W1, b1, W2, b2):
    n = np.asarray(n_instances, dtype=np.int64)
    assignment = _plan(n)[0]
    in_maps = make_in_maps(x, n_instances, W1, b1, W2, b2)
    res = run_spmd(in_maps)
    out = np.empty((B, 1), dtype=np.float32)
    for c in range(N_CORES):
        out[assignment[c]] = np.asarray(res.results[c]["out"],
                                        dtype=np.float32).reshape(BAGS, 1)
    return out


# revision 15
# speedup vs baseline: 1.2620x; 1.0307x over previous
"""Trainium2 Bass kernel for nn_BagModel_3d (segment_reduce).

Computation (per bag b):
  out[b] = (1/n_b) * sum_{i < n_b} relu(x[b, i, :] @ W1 + b1) @ W2 + b2

Strategy (8 cores, data-parallel over bags):
  * Host: sort bags by n_instances, snake-deal across cores (balanced work),
    concatenate ONLY the valid instances per core (exact compaction - the
    random n_b average ~256/512, so this halves DMA and matmul work), cast
    to bf16, zero-pad to G*128 columns.
  * Device, instance-major layout: for each 128-instance group, the x tile
    [d_in=128, inst=128] is the matmul STATIONARY operand and W1 [128, 256]
    the moving one, giving z^T [inst, dh] in PSUM.  The PSUM drain is then a
    bag-agnostic relu at FD=1024 (alternating ScalarE / VectorE) into a bf16
    h^T in SBUF - no per-bag accum pieces on the slow engines.
  * The ragged per-bag segment sum runs on TensorE: a {0,1} indicator matrix
    S[g] [128 inst, 32 bags] as stationary, h^T as moving, 4-way column
    tiling (tile_position), accumulating 4 PSUM band rows across all groups.
  * Final: one tensor_tensor_reduce contracts the pooled bands with W2 along
    the free axis, a [128,32] fold matmul adds the 4 bands per bag, then
    scale by 1/n_b and add b2 in a single tensor_scalar.
  * PE warmup: dummy matmuls during the initial DMA window so the HAM clock
    gate reaches 2.4 GHz before the real matmul stream starts.

b1 general-path note: padded columns are zeros and excluded by S, so no
relu(b1) correction is ever needed.  A nonzero b1 is folded in via an extra
K=1 matmul per group (ones-row x b1-row); the spec's b1 is all-zero so the
compiled program skips it.
"""
import os
import sys
import numpy as np

sys.path.insert(0, '/opt/trn_rl_repo')

# debug knobs (default = full-featured kernel)
DBG_NOWARM = os.environ.get('KDBG_NOWARM', '0') == '1'
DBG_NOTILEPOS = os.environ.get('KDBG_NOTILEPOS', '0') == '1'
DBG_NOPASS2 = os.environ.get('KDBG_NOPASS2', '0') == '1'
DBG_ALLSCALAR = os.environ.get('KDBG_ALLSCALAR', '0') == '1'
DBG_NOFINAL = os.environ.get('KDBG_NOFINAL', '0') == '1'
DBG_NODRAIN = os.environ.get('KDBG_NODRAIN', '0') == '1'

B, N_MAX, D_IN, D_H = 256, 512, 256, 256
N_CORES = 8
BAGS = B // N_CORES          # 32 bag slots per core
GPB = 4                      # instance groups per PSUM buffer
N_WARM = 10                  # warmup matmuls to bridge DMA wait
NB16 = 2                     # tail blocks kept in bf16 (smallest bags)

_PROGRAMS = {}


def _build_program(G, G8, b1_nonzero):
    import concourse.bacc as bacc
    import concourse.tile as tile
    from concourse import mybir

    f32 = mybir.dt.float32
    bf16 = mybir.dt.bfloat16
    fp8 = mybir.dt.float8e4
    Alu = mybir.AluOpType
    Act = mybir.ActivationFunctionType

    W = G * 128
    W8 = G8 * 128
    W16 = W - W8
    NBLK = G // GPB

    nc = bacc.Bacc("TRN2", target_bir_lowering=False, debug=False)

    xt = nc.dram_tensor("xt", [D_IN, W8], fp8, kind="ExternalInput").ap()
    xt16 = (nc.dram_tensor("xt16", [D_IN, W16], bf16, kind="ExternalInput").ap()
            if W16 else None)
    w1 = nc.dram_tensor("w1", [128, 2 * D_H], bf16, kind="ExternalInput").ap()
    s_t = nc.dram_tensor("s_t", [128, G * BAGS], bf16, kind="ExternalInput").ap()
    w2b = nc.dram_tensor("w2b", [128, D_H], f32, kind="ExternalInput").ap()
    fold = nc.dram_tensor("fold", [128, BAGS], f32, kind="ExternalInput").ap()
    inv = nc.dram_tensor("inv", [BAGS, 1], f32, kind="ExternalInput").ap()
    bias2 = nc.dram_tensor("bias2", [BAGS, 1], f32, kind="ExternalInput").ap()
    if b1_nonzero:
        b1r = nc.dram_tensor("b1r", [1, D_H], bf16, kind="ExternalInput").ap()
    out = nc.dram_tensor("out", [BAGS, 1], f32, kind="ExternalOutput").ap()

    with tile.TileContext(nc) as tc:
        with (
            tc.tile_pool(name="const", bufs=1) as cpool,
            tc.tile_pool(name="xsb", bufs=1) as xpool,
            tc.tile_pool(name="hsb", bufs=1) as hpool,
            tc.tile_pool(name="z", bufs=3, space="PSUM") as zpool,
            tc.tile_pool(name="sps", bufs=1, space="PSUM") as spool,
        ):
            # ---- PE warmup: zeros matmuls fill the HAM activity window ----
            warm = cpool.tile([128, 256], bf16, tag="warm")
            nc.vector.memset(warm[:], 0.0)
            small = spool.tile([128, 512], f32, tag="small")
            bands = small[:, 0:D_H]            # 4 x 32 bag-band rows
            pot = spool.tile([BAGS, 1], f32, tag="pot")
            po = pot[:]
            for _ in range(0 if DBG_NOWARM else N_WARM):
                nc.tensor.matmul(small[:, 256:512], warm[:, 0:128], warm[:],
                                 start=True, stop=True, skip_group_check=True)

            # ---- constants + x prefetch (HWDGE, bf16 host-cast) ----
            w1cat = cpool.tile([128, 2 * D_H], bf16, tag="w1cat")
            nc.sync.dma_start(w1cat[:], w1[:])
            w1k0 = w1cat[:, 0:D_H]
            w1k1 = w1cat[:, D_H:2 * D_H]
            xk0 = xpool.tile([128, max(W8, 128)], fp8, tag="xk0")
            xk1 = xpool.tile([128, max(W8, 128)], fp8, tag="xk1")
            s_sb = cpool.tile([128, G * BAGS], bf16, tag="s_sb")
            gb = [0, 2 * 128, 6 * 128, 12 * 128, 20 * 128, 30 * 128,
                  42 * 128, W8]
            bounds = sorted(set(min(b, W8) for b in gb))
            for ci in range(len(bounds) - 1):
                sl = slice(bounds[ci], bounds[ci + 1])
                if sl.start >= sl.stop:
                    continue
                nc.sync.dma_start(xk0[:, sl], xt[0:128, sl])
                nc.scalar.dma_start(xk1[:, sl], xt[128:256, sl])
                if ci == 2:
                    nc.sync.dma_start(s_sb[:], s_t[:])
            if W16:
                xk16_0 = xpool.tile([128, W16], bf16, tag="xk16_0")
                xk16_1 = xpool.tile([128, W16], bf16, tag="xk16_1")
                nc.scalar.dma_start(xk16_0[:], xt16[0:128, :])
                nc.scalar.dma_start(xk16_1[:], xt16[128:256, :])
            w2sb = cpool.tile([128, D_H], f32, tag="w2sb")
            nc.gpsimd.dma_start(w2sb[:], w2b[:])
            foldsb = cpool.tile([128, BAGS], f32, tag="foldsb")
            nc.gpsimd.dma_start(foldsb[:], fold[:])
            invsb = cpool.tile([BAGS, 1], f32, tag="invsb")
            nc.gpsimd.dma_start(invsb[:], inv[:])
            b2sb = cpool.tile([BAGS, 1], f32, tag="b2sb")
            nc.gpsimd.dma_start(b2sb[:], bias2[:])
            if b1_nonzero:
                onesr = cpool.tile([1, 128], bf16, tag="onesr")
                nc.vector.memset(onesr[:], 16.0)
                onesr1 = cpool.tile([1, 128], bf16, tag="onesr1")
                nc.vector.memset(onesr1[:], 1.0)
                b1sb = cpool.tile([1, D_H], bf16, tag="b1sb")
                nc.sync.dma_start(b1sb[:], b1r[:])

            hT = hpool.tile([128, G * D_H], bf16, tag="hT")

            # ---- main loop: GPB instance groups per PSUM buffer ----
            for blk in range(NBLK):
                zb = zpool.tile([128, GPB * D_H], f32, tag="zb")
                is8 = blk < G8 // GPB
                for j in range(GPB):
                    g = GPB * blk + j
                    zsl = zb[:, D_H * j:D_H * (j + 1)]
                    if is8:
                        xsl = slice(128 * g, 128 * (g + 1))
                        a0, a1 = xk0[:, xsl], xk1[:, xsl]
                    else:
                        xsl = slice(128 * (g - G8), 128 * (g - G8 + 1))
                        a0, a1 = xk16_0[:, xsl], xk16_1[:, xsl]
                    nc.tensor.matmul(zsl, a0, w1k0,
                                     start=True, stop=False)
                    nc.tensor.matmul(zsl, a1, w1k1,
                                     start=False, stop=not b1_nonzero)
                    if b1_nonzero:
                        nc.tensor.matmul(zsl, onesr[:] if is8 else onesr1[:],
                                         b1sb[:], start=False, stop=True)
                # relu drain, whole buffer in one big-FD instruction
                hsl = hT[:, GPB * D_H * blk:GPB * D_H * (blk + 1)]
                if DBG_NODRAIN:
                    pass
                elif DBG_ALLSCALAR or blk % 2 == 1:
                    nc.scalar.activation(hsl, zb[:], Act.Relu,
                                         bias=0.0, scale=(1.0 / 16.0) if is8 else 1.0)
                else:
                    nc.vector.tensor_scalar(hsl, zb[:], (1.0 / 16.0) if is8 else 1.0,
                                            0.0, op0=Alu.mult, op1=Alu.max)
                # segment-sum matmuls: indicator stationary, 4-way col tiling
                if not DBG_NOPASS2:
                    for j in range(GPB):
                        g = GPB * blk + j
                        jt = j % 4
                        kw = {}
                        if not DBG_NOTILEPOS:
                            kw['tile_position'] = (0, 32 * jt)
                        nc.tensor.matmul(
                            bands[32 * jt:32 * (jt + 1), :],
                            s_sb[:, BAGS * g:BAGS * (g + 1)],
                            hT[:, D_H * g:D_H * (g + 1)],
                            start=(g < 4), stop=(g >= G - 4),
                            skip_group_check=True, **kw)

            # ---- W2 contraction + band fold + mean + bias ----
            scr = cpool.tile([128, D_H], f32, tag="scr")
            acc = cpool.tile([128, 1], f32, tag="acc")
            osb = cpool.tile([BAGS, 1], f32, tag="osb")
            if DBG_NOFINAL:
                nc.vector.memset(osb[:], 0.0)
            else:
                # acc[p] = sum_dh bands[p, dh] * W2[dh]  (proven stt+accum form)
                nc.vector.scalar_tensor_tensor(
                    scr[:], bands, 0.0, w2sb[:], op0=Alu.add, op1=Alu.mult,
                    accum_out=acc[:])
                nc.tensor.matmul(po, foldsb[:], acc[:], start=True, stop=True,
                                 skip_group_check=True)
                nc.vector.tensor_scalar(osb[:], po, invsb[:, 0:1],
                                        b2sb[:, 0:1], op0=Alu.mult,
                                        op1=Alu.add)
            nc.sync.dma_start(out[:], osb[:])

    nc.compile()
    return nc


def get_program(G, G8, b1_nonzero):
    key = (int(G), int(G8), bool(b1_nonzero))
    if key not in _PROGRAMS:
        _PROGRAMS[key] = _build_program(*key)
    return _PROGRAMS[key]


def _plan(n):
    """Snake-deal bags (sorted by size, desc) across cores; return
    assignment[core][slot] -> bag id and G (shared group count)."""
    order = np.argsort(-n, kind='stable')
    assignment = np.empty((N_CORES, BAGS), dtype=np.int64)
    for i, bag in enumerate(order):
        r, p = divmod(i, N_CORES)
        core = p if (r % 2 == 0) else (N_CORES - 1 - p)
        assignment[core, r] = bag
    v_max = max(int(n[assignment[c]].sum()) for c in range(N_CORES))
    G = -(-v_max // 128)
    G = -(-G // GPB) * GPB          # multiple of GPB (psum buffer / col tiles)
    G8 = max(G - NB16 * GPB, 0)     # leading fp8 region; bf16 tail
    return assignment, G, G8


def make_in_maps(x, n_instances, W1, b1, W2, b2=None):
    import ml_dtypes
    bf16 = ml_dtypes.bfloat16
    fp8 = ml_dtypes.float8_e4m3

    x = np.asarray(x, dtype=np.float32)
    n = np.asarray(n_instances, dtype=np.int64)
    W1 = np.asarray(W1, dtype=np.float32)
    b1 = np.asarray(b1, dtype=np.float32).reshape(-1)
    W2 = np.asarray(W2, dtype=np.float32).reshape(-1)
    b2v = float(np.asarray(b2).reshape(-1)[0]) if b2 is not None else 0.0

    assignment, G, G8 = _plan(n)
    W = G * 128
    W8 = G8 * 128
    b1_nonzero = bool(np.any(b1 != 0.0))

    xflat = x.reshape(B * N_MAX, D_IN)
    w1_bf = np.ascontiguousarray(
        np.concatenate([W1[0:128, :], W1[128:256, :]], axis=1).astype(bf16))
    w2b = np.ascontiguousarray(
        np.broadcast_to(W2.reshape(1, D_H), (128, D_H)).astype(np.float32))
    foldm = np.zeros((128, BAGS), dtype=np.float32)
    foldm[np.arange(128), np.arange(128) % BAGS] = 1.0

    in_maps = []
    for c in range(N_CORES):
        bags = assignment[c]
        ns = n[bags]
        v = int(ns.sum())
        # gather valid instance rows: bag-major, instance-minor
        idx = np.concatenate(
            [bags[s] * N_MAX + np.arange(ns[s]) for s in range(BAGS)])
        xfull = np.zeros((D_IN, W), dtype=np.float32)
        xfull[:, :v] = xflat[idx].T
        xt = (xfull[:, :W8] * np.float32(16.0)).astype(fp8)
        xt16 = xfull[:, W8:].astype(bf16)
        # indicator S: [W, 32] -> [128, G*32]
        starts = np.zeros(BAGS + 1, dtype=np.int64)
        np.cumsum(ns, out=starts[1:])
        s_full = np.zeros((W, BAGS), dtype=bf16)
        for s in range(BAGS):
            s_full[starts[s]:starts[s + 1], s] = bf16(1.0)
        s_t = np.ascontiguousarray(
            s_full.reshape(G, 128, BAGS).transpose(1, 0, 2).reshape(128, G * BAGS))
        im = {
            "xt": xt,
            "xt16": xt16,
            "w1": w1_bf,
            "s_t": s_t,
            "w2b": w2b,
            "fold": foldm,
            "inv": (1.0 / ns.astype(np.float32)).reshape(BAGS, 1),
            "bias2": np.full((BAGS, 1), b2v, dtype=np.float32),
        }
        if b1_nonzero:
            im["b1r"] = np.ascontiguousarray(b1.reshape(1, D_H).astype(bf16))
        in_maps.append(im)
    return in_maps


def run_spmd(in_maps, b2_value=0.0, trace=False, **kwargs):
    from concourse import bass_utils
    if trace:
        # no S3 in this environment; keep trace artifacts local
        bass_utils.upload_artifacts = lambda tmpdir: tmpdir
    G8 = in_maps[0]["xt"].shape[1] // 128
    G = G8 + in_maps[0]["xt16"].shape[1] // 128
    nc = get_program(G, G8, "b1r" in in_maps[0])
    return bass_utils.run_bass_kernel_spmd(
        nc, in_maps, core_ids=list(range(N_CORES)), trace=trace, **kwargs)


def kernel(x, n_instances, W1, b1, W2, b2):
    n = np.asarray(n_instances, dtype=np.int64)
    assignment = _plan(n)[0]
    in_maps = make_in_maps(x, n_instances, W1, b1, W2, b2)
    res = run_spmd(in_maps)
    out = np.empty((B, 1), dtype=np.float32)
    for c in range(N_CORES):
        out[assignment[c]] = np.asarray(res.results[c]["out"],
                                        dtype=np.float32).reshape(BAGS, 1)
    return out
